# revision 12
# baseline (speedup 1.0000x reference)
"""GwcVolume (group-wise correlation cost volume) Trainium2 kernel.

cost[b,g,d,h,x] = mean_c( lf[b, g*8+c, h, x] * rf[b, g*8+c, h, x-d] ),
zero for x < d.  Shapes: lf/rf [2, 320, 128, 240] f32 -> out [2, 40, 48, 128, 240] f32.

Sharding: h-axis split across 8 cores (16 rows each). Correlation is along w
only, so shards are fully independent and each core reads just its h-band.

Per-core design (v2):
  - Channels split into five 128-partition product slabs: A0/B0 = b0 chans
    0-127/128-255, A1/B1 = same for b1, C01 = chans 256-319 of both batches
    stacked (64+64). Every elementwise multiply runs on full 128 partitions.
  - SWDGE DMA loads the h-band casting fp32->fp16. rf is stored twice (48-
    and 49-column zero left-pad) so every disparity window starts 4B-aligned
    and DVE 2x perf mode always engages; the pad also zeroes x<d for free.
  - All five slabs live in one [128, 40, W] SBUF tile (5 chunks x 8 h rows
    on the free axis). Per (hg, d) one DVE tensor_mul covers rows [0:SPLIT]
    (2-dim free AP keeps the DVE 2x fp16 perf mode engaged) while the
    Pool/GpSimd engine multiplies rows [SPLIT:40] in parallel (measured
    ~3.7 ns/elem there, so it carries ~1/8 of the rows).
  - TensorE reduces channel groups via 0/1*(1/8) block-diagonal stationaries.
    PSUM tile positions allow only 4 column strips of 16; C01 is folded into
    strip 0 with a 32-wide stationary (cols 16-31) using start/stop
    accumulation flags, so 5 slabs fit in 4 positions at zero extra PE rows.
  - x < d output is zero; products are only computed for x >= e, e = d&~1
    (even so 4B alignment survives). Host zeroes x < e.
  - ScalarE drains PSUM->SBUF casting to fp16; HWDGE writes contiguous
    [hg,d,80,8,240] fp16 output; host reassembles + casts to fp32.
"""

import numpy as np

import concourse.bass as bass
import concourse.tile as tile
from concourse import mybir
from concourse.bass_utils import run_bass_kernel_spmd

B = 2
C = 320
H = 128
W = 240
G = 40
CPG = 8
D = 48
NCORES = 8
HS = H // NCORES  # h rows per core
HB = 8  # h rows per inner block
PW = 0  # rf window always starts at col PW; only rf_ao col 0 is ever
     # read as zero (odd d, x = d-1), so no wide left-pad is needed
F16 = mybir.dt.float16
F32 = mybir.dt.float32

# big-slab chunk index -> (batch, channel offset)
BIG_CHUNKS = [(0, 0), (0, 128), (1, 0), (1, 128)]
# psum strips: [0:16]=A0, [16:32]=C01, [32:48]=B0, [64:80]=A1, [96:112]=B1
STRIP = {0: 0, 1: 32, 2: 64, 3: 96}
NROW = 40  # 5 chunks x 8 h rows stacked on the free axis
SPLIT = 40  # rows [0:SPLIT] multiply on DVE, rest (if any) on Pool/GpSimd.
# SPLIT=40: DVE does everything. GpSimd tensor ops CONTEND with DVE for
# SBUF ports (measured: DVE drops from ~2.0 to ~1.45 elem/cyc while a
# GpSimd tensor_tensor streams), so offloading 5/40 rows was a net loss.


def split_multi_waits(nc, limit=1):
    """Walrus in this container rejects instructions carrying more than
    `limit` semaphore waits. Move excess waits onto preceding NoOps on the
    same engine (waits execute before the instruction, in stream order)."""
    n_split = 0
    for fn in nc.m.functions:
        for bb in fn.blocks:
            insts = bb.instructions
            i = 0
            while i < len(insts):
                inst = insts[i]
                si = inst.sync_info
                if si is not None and len(si.on_wait) > limit:
                    waits = list(si.on_wait)
                    keep = waits[-limit:]
                    extra = waits[:-limit]
                    new_insts = []
                    for j in range(0, len(extra), limit):
                        chunk = extra[j : j + limit]
                        nop = mybir.InstNoOp(
                            name=nc.get_next_instruction_name(),
                            engine=inst.engine,
                            ins=[],
                            outs=[],
                            sync_info=mybir.SyncInfo(on_wait=chunk, on_update=[]),
                        )
                        new_insts.append(nop)
                    inst.sync_info = mybir.SyncInfo(
                        on_wait=keep, on_update=list(si.on_update)
                    )
                    insts[i:i] = new_insts
                    i += len(new_insts)
                    n_split += 1
                i += 1
    return n_split


def build_bass(n_hb=HS // HB, n_d=D, zero_skip=True):
    nc = bass.Bass("TRN2", target_bir_lowering=False, debug=False, num_devices=NCORES)
    lf = nc.dram_tensor("lf", [B, C, HS, W], F32, kind="ExternalInput").ap()
    rf = nc.dram_tensor("rf", [B, C, HS, W], F32, kind="ExternalInput").ap()
    # sL: cols 0-15 block-diag 1/8 over 128 chans (16 groups); cols 16-31 zero.
    # sC: cols 0-15 zero; cols 16-23 groups of partitions 0-63, 24-31 of 64-127.
    sL = nc.dram_tensor("sL", [128, 32], F16, kind="ExternalInput").ap()
    sC = nc.dram_tensor("sC", [128, 32], F16, kind="ExternalInput").ap()
    outp = nc.dram_tensor("outp", [n_hb, n_d, 80, HB, W], F16, kind="ExternalOutput").ap()

    with tile.TileContext(nc) as tc:
        with (
            tc.tile_pool(name="const", bufs=1) as cpool,
            tc.tile_pool(name="loads", bufs=2) as lpool,
            tc.tile_pool(name="prod", bufs=2) as ppool,
            tc.tile_pool(name="outs", bufs=3) as opool,
            tc.tile_pool(name="psum", bufs=2, space="PSUM") as qpool,
        ):
            sL_t = cpool.tile([128, 32], F16)
            nc.sync.dma_start(sL_t[:], sL[:])
            sC_t = cpool.tile([128, 32], F16)
            nc.sync.dma_start(sC_t[:], sC[:])

            for hg in range(n_hb):
                h0 = hg * HB
                # ---- loads (SWDGE, fp32->fp16 cast in flight) ----
                # rows r = 8*ci + h: ci 0-3 = A0/B0/A1/B1 (128 chans of one
                # batch), ci 4 = C01 (chans 256-319, b in partition halves)
                lf_all = lpool.tile([128, NROW, W], F16, tag="lfall")
                rf_ae = lpool.tile([128, NROW, W], F16, tag="rfe")
                rf_ao = lpool.tile([128, NROW, W + 2], F16, tag="rfo")
                nc.scalar.memzero(rf_ao[:, :, 0:2])
                # chunk-major so compute can chase the loads; rf read from HBM
                # once (ae), the odd-aligned copy is a cheap SBUF->SBUF DMA
                for ci, (b, c0) in enumerate(BIG_CHUNKS):
                    r0 = 8 * ci
                    nc.gpsimd.dma_start(
                        lf_all[:, r0 : r0 + HB, :], lf[b, c0 : c0 + 128, h0 : h0 + HB, :]
                    )
                    nc.gpsimd.dma_start(
                        rf_ae[:, r0 : r0 + HB, PW : PW + W],
                        rf[b, c0 : c0 + 128, h0 : h0 + HB, :],
                    )
                    nc.sync.dma_start(
                        rf_ao[:, r0 : r0 + HB, PW + 1 : PW + 1 + W],
                        rf_ae[:, r0 : r0 + HB, PW : PW + W],
                    )
                for b in range(2):
                    p0 = 64 * b
                    nc.gpsimd.dma_start(
                        lf_all[p0 : p0 + 64, 32:NROW, :], lf[b, 256:320, h0 : h0 + HB, :]
                    )
                    nc.gpsimd.dma_start(
                        rf_ae[p0 : p0 + 64, 32:NROW, PW : PW + W],
                        rf[b, 256:320, h0 : h0 + HB, :],
                    )
                    nc.sync.dma_start(
                        rf_ao[p0 : p0 + 64, 32:NROW, PW + 1 : PW + 1 + W],
                        rf_ae[p0 : p0 + 64, 32:NROW, PW : PW + W],
                    )

                for d in range(n_d):
                    e = (d & ~1) if zero_skip else 0
                    wlen = W - e
                    # rf window start within the padded tile (always even):
                    # even d: pad PW,   start = PW - d + e = PW (d even, e=d)
                    # odd d:  pad PW+1, start = PW + 1 - d + e = PW
                    rfa = rf_ae if d % 2 == 0 else rf_ao
                    # ---- multiplies: rows [0:SPLIT] on DVE, rest on Pool ----
                    pr = ppool.tile([128, NROW, W], F16, tag="pr")
                    if hg == 0 and d < 2:
                        # per-chunk sub-ops so the multiply chases the loads
                        for ci in range(5):
                            r0 = 8 * ci
                            nc.vector.tensor_mul(
                                pr[:, r0 : r0 + HB, e:W],
                                lf_all[:, r0 : r0 + HB, e:W],
                                rfa[:, r0 : r0 + HB, PW : PW + wlen],
                            )
                    else:
                        nc.vector.tensor_mul(
                            pr[:, 0:SPLIT, e:W],
                            lf_all[:, 0:SPLIT, e:W],
                            rfa[:, 0:SPLIT, PW : PW + wlen],
                        )
                        if SPLIT < NROW:
                            nc.gpsimd.tensor_mul(
                                pr[:, SPLIT:NROW, e:W],
                                lf_all[:, SPLIT:NROW, e:W],
                                rfa[:, SPLIT:NROW, PW : PW + wlen],
                            )
                    # ---- reduce 8 chans -> group via PE ----
                    ps = qpool.tile([112, HB, 256], F32)
                    for j in range(HB // 2):
                        hsl = slice(2 * j, 2 * j + 2)
                        # strip 0: A0 (start) then C01 (accumulate, stop)
                        nc.tensor.matmul(
                            ps[0:32, hsl, e:W],
                            sL_t[:, :],
                            pr[:, 2 * j : 2 * j + 2, e:W],
                            start=True,
                            stop=False,
                            tile_position=(0, 0),
                        )
                        nc.tensor.matmul(
                            ps[0:32, hsl, e:W],
                            sC_t[:, :],
                            pr[:, 32 + 2 * j : 32 + 2 * j + 2, e:W],
                            start=False,
                            stop=True,
                            tile_position=(0, 0),
                        )
                        for ci in (1, 2, 3):
                            st = STRIP[ci]
                            r0 = 8 * ci
                            nc.tensor.matmul(
                                ps[st : st + 16, hsl, e:W],
                                sL_t[:, 0:16],
                                pr[:, r0 + 2 * j : r0 + 2 * j + 2, e:W],
                                start=True,
                                stop=True,
                                tile_position=(0, st),
                            )
                    # ---- drain PSUM -> SBUF (cast fp16) ----
                    ot = opool.tile([112, HB, W], F16)
                    nc.scalar.copy(ot[:, :, e:W], ps[:, :, e:W])
                    # ---- store: strips {0:48}, {64:80}, {96:112} ----
                    nc.sync.dma_start(
                        outp[hg, d, 0:48, :, e:W], ot[0:48, :, e:W]
                    )
                    nc.sync.dma_start(
                        outp[hg, d, 48:64, :, e:W], ot[64:80, :, e:W]
                    )
                    nc.sync.dma_start(
                        outp[hg, d, 64:80, :, e:W], ot[96:112, :, e:W]
                    )
    split_multi_waits(nc)
    return nc


def make_smats():
    sL = np.zeros((128, 32), np.float16)
    for g in range(16):
        sL[g * CPG : (g + 1) * CPG, g] = 1.0 / CPG
    sC = np.zeros((128, 32), np.float16)
    for g in range(16):
        sC[g * CPG : (g + 1) * CPG, 16 + g] = 1.0 / CPG
    return sL, sC


# device psum strip p (0..79) -> (batch, group) in the full output
def strip_perm():
    # [0:16]=b0 g0-15, [16:24]=b0 g32-39, [24:32]=b1 g32-39,
    # [32:48]=b0 g16-31, [48:64]=b1 g0-15, [64:80]=b1 g16-31
    perm = np.empty(80, np.int64)
    perm[0:16] = 0 * 40 + np.arange(0, 16)
    perm[16:24] = 0 * 40 + np.arange(32, 40)
    perm[24:32] = 1 * 40 + np.arange(32, 40)
    perm[32:48] = 0 * 40 + np.arange(16, 32)
    perm[48:64] = 1 * 40 + np.arange(0, 16)
    perm[64:80] = 1 * 40 + np.arange(16, 32)
    return perm


_NC_CACHE = {}


def _get_nc(key=(HS // HB, D)):
    if key not in _NC_CACHE:
        _NC_CACHE[key] = build_bass(*key)
    return _NC_CACHE[key]


def run_sharded(lf, rf, nc=None, trace=False, tmpdir=None, n_hb=HS // HB, n_d=D):
    """lf/rf: full [2, 320, 128, 240] f32 numpy arrays. Returns (out, results)."""
    if nc is None:
        nc = _get_nc()
    sL, sC = make_smats()
    in_maps = []
    for k in range(NCORES):
        in_maps.append(
            {
                "lf": np.ascontiguousarray(lf[:, :, k * HS : (k + 1) * HS, :]),
                "rf": np.ascontiguousarray(rf[:, :, k * HS : (k + 1) * HS, :]),
                "sL": sL,
                "sC": sC,
            }
        )
    res = run_bass_kernel_spmd(
        nc, in_maps, list(range(NCORES)), trace=trace, tmpdir=tmpdir
    )
    perm = strip_perm()
    inv = np.argsort(perm)  # (b*40+g) -> device strip index
    out = np.zeros((B, G, D, H, W), np.float32)
    for k in range(NCORES):
        dev = res.results[k]["outp"]  # [n_hb, n_d, 80, HB, W] fp16
        # -> [80(bg), n_d, n_hb, HB, W] then [b, g, d, h_band, W]
        o = dev.transpose(2, 1, 0, 3, 4)[inv].astype(np.float32)
        o = o.reshape(B, G, n_d, n_hb * HB, W)
        out[:, :, :n_d, k * HS : k * HS + n_hb * HB, :] = o
    # x < e was never written by the device; force the zero region clean
    for d in range(n_d):
        e = d & ~1
        if e:
            out[:, :, d, :, :e] = 0.0
    return out, res


def kernel(**inputs):
    lf = np.asarray(inputs["left_feature"], dtype=np.float32)
    rf = np.asarray(inputs["right_feature"], dtype=np.float32)
    out, _ = run_sharded(lf, rf)
    return out


if __name__ == "__main__":
    rng = np.random.default_rng(0)
    lf = rng.standard_normal((B, C, H, W), dtype=np.float32)
    rf = rng.standard_normal((B, C, H, W), dtype=np.float32)
    out, _ = run_sharded(lf, rf)
    print(out.shape, out.dtype, float(np.abs(out).max()))


# revision 14
# speedup vs baseline: 1.0990x; 1.0990x over previous
"""GwcVolume (group-wise correlation cost volume) Trainium2 kernel.

cost[b,g,d,h,x] = mean_c( lf[b, g*8+c, h, x] * rf[b, g*8+c, h, x-d] ),
zero for x < d.  Shapes: lf/rf [2, 320, 128, 240] f32 -> out [2, 40, 48, 128, 240] f32.

Sharding: h-axis split across 8 cores (16 rows each). Correlation is along w
only, so shards are fully independent and each core reads just its h-band.

Per-core design (v2):
  - Channels split into five 128-partition product slabs: A0/B0 = b0 chans
    0-127/128-255, A1/B1 = same for b1, C01 = chans 256-319 of both batches
    stacked (64+64). Every elementwise multiply runs on full 128 partitions.
  - SWDGE DMA loads the h-band casting fp32->fp16. rf is stored twice (48-
    and 49-column zero left-pad) so every disparity window starts 4B-aligned
    and DVE 2x perf mode always engages; the pad also zeroes x<d for free.
  - All five slabs live in one [128, 40, W] SBUF tile (5 chunks x 8 h rows
    on the free axis). Per (hg, d) one DVE tensor_mul covers rows [0:SPLIT]
    (2-dim free AP keeps the DVE 2x fp16 perf mode engaged) while the
    Pool/GpSimd engine multiplies rows [SPLIT:40] in parallel (measured
    ~3.7 ns/elem there, so it carries ~1/8 of the rows).
  - TensorE reduces channel groups via 0/1*(1/8) block-diagonal stationaries.
    PSUM tile positions allow only 4 column strips of 16; C01 is folded into
    strip 0 with a 32-wide stationary (cols 16-31) using start/stop
    accumulation flags, so 5 slabs fit in 4 positions at zero extra PE rows.
  - x < d output is zero; products are only computed for x >= e, e = d&~1
    (even so 4B alignment survives). Host zeroes x < e.
  - ScalarE drains PSUM->SBUF casting to fp16; HWDGE writes contiguous
    [hg,d,80,8,240] fp16 output; host reassembles + casts to fp32.
"""

import numpy as np

import concourse.bass as bass
import concourse.tile as tile
from concourse import mybir
from concourse.bass_utils import run_bass_kernel_spmd

B = 2
C = 320
H = 128
W = 240
G = 40
CPG = 8
D = 48
NCORES = 8
HS = H // NCORES  # h rows per core
HB = 8  # h rows per inner block
PW = 0  # rf window always starts at col PW; only rf_ao col 0 is ever
     # read as zero (odd d, x = d-1), so no wide left-pad is needed
F16 = mybir.dt.float16
F32 = mybir.dt.float32

# big-slab chunk index -> (batch, channel offset)
BIG_CHUNKS = [(0, 0), (0, 128), (1, 0), (1, 128)]
# psum strips: [0:16]=A0, [16:32]=C01, [32:48]=B0, [64:80]=A1, [96:112]=B1
STRIP = {0: 0, 1: 32, 2: 64, 3: 96}
NROW = 40  # 5 chunks x 8 h rows stacked on the free axis
SPLIT = 40  # rows [0:SPLIT] multiply on DVE, rest (if any) on Pool/GpSimd.
# SPLIT=40: DVE does everything. GpSimd tensor ops CONTEND with DVE for
# SBUF ports (measured: DVE drops from ~2.0 to ~1.45 elem/cyc while a
# GpSimd tensor_tensor streams), so offloading 5/40 rows was a net loss.


def split_multi_waits(nc, limit=1):
    """Walrus in this container rejects instructions carrying more than
    `limit` semaphore waits. Move excess waits onto preceding NoOps on the
    same engine (waits execute before the instruction, in stream order)."""
    n_split = 0
    for fn in nc.m.functions:
        for bb in fn.blocks:
            insts = bb.instructions
            i = 0
            while i < len(insts):
                inst = insts[i]
                si = inst.sync_info
                if si is not None and len(si.on_wait) > limit:
                    waits = list(si.on_wait)
                    keep = waits[-limit:]
                    extra = waits[:-limit]
                    new_insts = []
                    for j in range(0, len(extra), limit):
                        chunk = extra[j : j + limit]
                        nop = mybir.InstNoOp(
                            name=nc.get_next_instruction_name(),
                            engine=inst.engine,
                            ins=[],
                            outs=[],
                            sync_info=mybir.SyncInfo(on_wait=chunk, on_update=[]),
                        )
                        new_insts.append(nop)
                    inst.sync_info = mybir.SyncInfo(
                        on_wait=keep, on_update=list(si.on_update)
                    )
                    insts[i:i] = new_insts
                    i += len(new_insts)
                    n_split += 1
                i += 1
    return n_split


def build_bass(n_hb=HS // HB, n_d=D, zero_skip=True):
    nc = bass.Bass("TRN2", target_bir_lowering=False, debug=False, num_devices=NCORES)
    lf = nc.dram_tensor("lf", [B, C, HS, W], F32, kind="ExternalInput").ap()
    rf = nc.dram_tensor("rf", [B, C, HS, W], F32, kind="ExternalInput").ap()
    # sL: cols 0-15 block-diag 1/8 over 128 chans (16 groups); cols 16-31 zero.
    # sC: cols 0-15 zero; cols 16-23 groups of partitions 0-63, 24-31 of 64-127.
    sL = nc.dram_tensor("sL", [128, 32], F16, kind="ExternalInput").ap()
    sC = nc.dram_tensor("sC", [128, 32], F16, kind="ExternalInput").ap()
    outp = nc.dram_tensor("outp", [n_hb, n_d, 80, HB, W], F16, kind="ExternalOutput").ap()

    with tile.TileContext(nc) as tc:
        with (
            tc.tile_pool(name="const", bufs=1) as cpool,
            tc.tile_pool(name="loads", bufs=2) as lpool,
            tc.tile_pool(name="prod", bufs=2) as ppool,
            tc.tile_pool(name="outs", bufs=3) as opool,
            tc.tile_pool(name="psum", bufs=2, space="PSUM") as qpool,
        ):
            sL_t = cpool.tile([128, 32], F16)
            nc.sync.dma_start(sL_t[:], sL[:])
            sC_t = cpool.tile([128, 32], F16)
            nc.sync.dma_start(sC_t[:], sC[:])

            for hg in range(n_hb):
                h0 = hg * HB
                # ---- loads (SWDGE, fp32->fp16 cast in flight) ----
                # rows r = 8*ci + h: ci 0-3 = A0/B0/A1/B1 (128 chans of one
                # batch), ci 4 = C01 (chans 256-319, b in partition halves)
                lf_all = lpool.tile([128, NROW, W], F16, tag="lfall")
                rf_ae = lpool.tile([128, NROW, W], F16, tag="rfe")
                rf_ao = lpool.tile([128, NROW, W + 2], F16, tag="rfo")
                nc.scalar.memzero(rf_ao[:, :, 0:2])
                # chunk-major so compute can chase the loads; rf read from HBM
                # once (ae), the odd-aligned copy is a cheap SBUF->SBUF DMA
                for ci, (b, c0) in enumerate(BIG_CHUNKS):
                    r0 = 8 * ci
                    nc.gpsimd.dma_start(
                        lf_all[:, r0 : r0 + HB, :], lf[b, c0 : c0 + 128, h0 : h0 + HB, :]
                    )
                    nc.gpsimd.dma_start(
                        rf_ae[:, r0 : r0 + HB, PW : PW + W],
                        rf[b, c0 : c0 + 128, h0 : h0 + HB, :],
                    )
                    nc.scalar.copy(
                        rf_ao[:, r0 : r0 + HB, PW + 1 : PW + 1 + W],
                        rf_ae[:, r0 : r0 + HB, PW : PW + W],
                    )
                for b in range(2):
                    p0 = 64 * b
                    nc.gpsimd.dma_start(
                        lf_all[p0 : p0 + 64, 32:NROW, :], lf[b, 256:320, h0 : h0 + HB, :]
                    )
                    nc.gpsimd.dma_start(
                        rf_ae[p0 : p0 + 64, 32:NROW, PW : PW + W],
                        rf[b, 256:320, h0 : h0 + HB, :],
                    )
                    nc.scalar.copy(
                        rf_ao[p0 : p0 + 64, 32:NROW, PW + 1 : PW + 1 + W],
                        rf_ae[p0 : p0 + 64, 32:NROW, PW : PW + W],
                    )

                # evens first: they only need rf_ae, so the odd-aligned copy
                # (ACT) finishes in their shadow and odd d's never stall
                for d in list(range(0, n_d, 2)) + list(range(1, n_d, 2)):
                    e = (d & ~1) if zero_skip else 0
                    wlen = W - e
                    # rf window start within the padded tile (always even):
                    # even d: pad PW,   start = PW - d + e = PW (d even, e=d)
                    # odd d:  pad PW+1, start = PW + 1 - d + e = PW
                    rfa = rf_ae if d % 2 == 0 else rf_ao
                    # ---- multiplies: rows [0:SPLIT] on DVE, rest on Pool ----
                    pr = ppool.tile([128, NROW, W], F16, tag="pr")
                    if hg == 0 and d in (0, 2):
                        # per-chunk sub-ops so the multiply chases the loads
                        for ci in range(5):
                            r0 = 8 * ci
                            nc.vector.tensor_mul(
                                pr[:, r0 : r0 + HB, e:W],
                                lf_all[:, r0 : r0 + HB, e:W],
                                rfa[:, r0 : r0 + HB, PW : PW + wlen],
                            )
                    else:
                        nc.vector.tensor_mul(
                            pr[:, 0:SPLIT, e:W],
                            lf_all[:, 0:SPLIT, e:W],
                            rfa[:, 0:SPLIT, PW : PW + wlen],
                        )
                        if SPLIT < NROW:
                            nc.gpsimd.tensor_mul(
                                pr[:, SPLIT:NROW, e:W],
                                lf_all[:, SPLIT:NROW, e:W],
                                rfa[:, SPLIT:NROW, PW : PW + wlen],
                            )
                    # ---- reduce 8 chans -> group via PE ----
                    ps = qpool.tile([112, HB, 256], F32)
                    for j in range(HB // 2):
                        hsl = slice(2 * j, 2 * j + 2)
                        # strip 0: A0 (start) then C01 (accumulate, stop)
                        nc.tensor.matmul(
                            ps[0:32, hsl, e:W],
                            sL_t[:, :],
                            pr[:, 2 * j : 2 * j + 2, e:W],
                            start=True,
                            stop=False,
                            tile_position=(0, 0),
                        )
                        nc.tensor.matmul(
                            ps[0:32, hsl, e:W],
                            sC_t[:, :],
                            pr[:, 32 + 2 * j : 32 + 2 * j + 2, e:W],
                            start=False,
                            stop=True,
                            tile_position=(0, 0),
                        )
                        for ci in (1, 2, 3):
                            st = STRIP[ci]
                            r0 = 8 * ci
                            nc.tensor.matmul(
                                ps[st : st + 16, hsl, e:W],
                                sL_t[:, 0:16],
                                pr[:, r0 + 2 * j : r0 + 2 * j + 2, e:W],
                                start=True,
                                stop=True,
                                tile_position=(0, st),
                            )
                    # ---- drain PSUM -> SBUF (cast fp16) ----
                    ot = opool.tile([112, HB, W], F16)
                    nc.scalar.copy(ot[:, :, e:W], ps[:, :, e:W])
                    # ---- store: strips {0:48}, {64:80}, {96:112} ----
                    nc.sync.dma_start(
                        outp[hg, d, 0:48, :, e:W], ot[0:48, :, e:W]
                    )
                    nc.sync.dma_start(
                        outp[hg, d, 48:64, :, e:W], ot[64:80, :, e:W]
                    )
                    nc.sync.dma_start(
                        outp[hg, d, 64:80, :, e:W], ot[96:112, :, e:W]
                    )
    split_multi_waits(nc)
    return nc


def make_smats():
    sL = np.zeros((128, 32), np.float16)
    for g in range(16):
        sL[g * CPG : (g + 1) * CPG, g] = 1.0 / CPG
    sC = np.zeros((128, 32), np.float16)
    for g in range(16):
        sC[g * CPG : (g + 1) * CPG, 16 + g] = 1.0 / CPG
    return sL, sC


# device psum strip p (0..79) -> (batch, group) in the full output
def strip_perm():
    # [0:16]=b0 g0-15, [16:24]=b0 g32-39, [24:32]=b1 g32-39,
    # [32:48]=b0 g16-31, [48:64]=b1 g0-15, [64:80]=b1 g16-31
    perm = np.empty(80, np.int64)
    perm[0:16] = 0 * 40 + np.arange(0, 16)
    perm[16:24] = 0 * 40 + np.arange(32, 40)
    perm[24:32] = 1 * 40 + np.arange(32, 40)
    perm[32:48] = 0 * 40 + np.arange(16, 32)
    perm[48:64] = 1 * 40 + np.arange(0, 16)
    perm[64:80] = 1 * 40 + np.arange(16, 32)
    return perm


_NC_CACHE = {}


def _get_nc(key=(HS // HB, D)):
    if key not in _NC_CACHE:
        _NC_CACHE[key] = build_bass(*key)
    return _NC_CACHE[key]


def run_sharded(lf, rf, nc=None, trace=False, tmpdir=None, n_hb=HS // HB, n_d=D):
    """lf/rf: full [2, 320, 128, 240] f32 numpy arrays. Returns (out, results)."""
    if nc is None:
        nc = _get_nc()
    sL, sC = make_smats()
    in_maps = []
    for k in range(NCORES):
        in_maps.append(
            {
                "lf": np.ascontiguousarray(lf[:, :, k * HS : (k + 1) * HS, :]),
                "rf": np.ascontiguousarray(rf[:, :, k * HS : (k + 1) * HS, :]),
                "sL": sL,
                "sC": sC,
            }
        )
    res = run_bass_kernel_spmd(
        nc, in_maps, list(range(NCORES)), trace=trace, tmpdir=tmpdir
    )
    perm = strip_perm()
    inv = np.argsort(perm)  # (b*40+g) -> device strip index
    out = np.zeros((B, G, D, H, W), np.float32)
    for k in range(NCORES):
        dev = res.results[k]["outp"]  # [n_hb, n_d, 80, HB, W] fp16
        # -> [80(bg), n_d, n_hb, HB, W] then [b, g, d, h_band, W]
        o = dev.transpose(2, 1, 0, 3, 4)[inv].astype(np.float32)
        o = o.reshape(B, G, n_d, n_hb * HB, W)
        out[:, :, :n_d, k * HS : k * HS + n_hb * HB, :] = o
    # x < e was never written by the device; force the zero region clean
    for d in range(n_d):
        e = d & ~1
        if e:
            out[:, :, d, :, :e] = 0.0
    return out, res


def kernel(**inputs):
    lf = np.asarray(inputs["left_feature"], dtype=np.float32)
    rf = np.asarray(inputs["right_feature"], dtype=np.float32)
    out, _ = run_sharded(lf, rf)
    return out


if __name__ == "__main__":
    rng = np.random.default_rng(0)
    lf = rng.standard_normal((B, C, H, W), dtype=np.float32)
    rf = rng.standard_normal((B, C, H, W), dtype=np.float32)
    out, _ = run_sharded(lf, rf)
    print(out.shape, out.dtype, float(np.abs(out).max()))
